# revision 1
# baseline (speedup 1.0000x reference)
"""DecorelationNormalization (training fwd) as a single SPMD Bass kernel on 8 TRN2 cores.

Math (reference): f = x viewed (c, n); m = mean(f); cov = (f-m)(f-m)^T/(n-1);
A = (1-eps)cov + eps I; L = chol(A); W = L^{-1}; out = W (f - m), back to NHWC.

Device algorithm:
  Phase 1  (per core, streaming x shard): convert tiles to bf16; Gram G = sum x x^T
           and s = sum x via one augmented matmul per tile (bf16 in, fp32 PSUM
           accumulate); also PE-transpose every bf16 tile into an SBUF-resident
           x^T copy for the whiten phase.
  Phase 2  AllReduce of [G | s] (256 x 257 fp32) across the 8 cores.
  Phase 3  A = c1*G - (c1/n) s s^T + eps I.
  Phase 4  Inverse-Cholesky via a triangular Newton iteration in fp32 (all matmuls):
              R   = W A W^T - I          (W lower-tri, V = W^T kept alongside)
              W  <- W - Phi(R) W,  V <- V - V Phi(R)^T
           with Phi(R) = strict_lower(R) + 0.5 diag(R).  Quadratic convergence
           to the exact inverse Cholesky factor (unique lower-tri fixed point
           with positive diagonal).  Seeded with W0 = I - Phi(A - I).
  Phase 5  mvb = broadcast of (s^T V)/n  (the mean correction row).
  Phase 6  out tile = x_tile @ V - mvb (bf16 matmul, fp32 PSUM), streamed from
           the resident x^T.

Sharding: data-parallel over samples; each core gets 4 of 32 batches
(16384 samples); only the 256x257 Gram is all-reduced.
"""

import os
import sys

import numpy as np

for _p in ("/opt/trn_rl_repo", "/root/.axon_site/_ro/trn_rl_repo"):
    if os.path.isdir(_p) and _p not in sys.path:
        sys.path.append(_p)

import concourse.bacc as bacc
import concourse.mybir as mybir
import concourse.tile as tile
from concourse.bass_utils import run_bass_kernel_spmd

EPS = 0.001
C = 256
P = 128
NCORES = 8
N_ITERS = 4
N_FP32 = 2  # trailing fp32 polish iterations
VTAG_LEN = 14  # bump on every kernel revision (forces HLO cache miss)
TP_FRAC_NUM, TP_FRAC_DEN = 5, 8  # fraction of transposes inlined in phase 1
JG = 8  # sample tiles per DMA group (1 MiB fp32 loads)
HGT = 8  # sample tiles per output store
STG_BUFS = 3
PST_BUFS = 4
PTR_BUFS = 2
NXA = 3  # rotating fp32 staging buffers
NXB = 4  # rotating bf16 buffers for inline-transposed groups
NG_DEF = 6  # trailing groups kept resident; transposes deferred

F32 = mybir.dt.float32
BF16 = mybir.dt.bfloat16
AL = mybir.AluOpType

CONST_NAMES = ("ml", "mu", "ih", "epsi", "c15", "eye128b", "onesrow")


def make_consts():
    import ml_dtypes

    i = np.eye(C, dtype=np.float32)
    return {
        "ml": np.tril(np.ones((C, C), np.float32), -1) + 0.5 * i,
        "mu": np.triu(np.ones((C, C), np.float32), 1) + 0.5 * i,
        "ih": 0.5 * i,
        "epsi": EPS * i,
        "c15": 1.5 * i,
        "eye128b": np.eye(P, dtype=ml_dtypes.bfloat16),
        "onesrow": np.ones((1, P), np.float32),
    }


def build(nloc: int, ncores: int = NCORES, n_iters: int | None = None,
          do_whiten: bool = True, do_gram: bool = True, do_tp: bool = True):
    """Build + compile the SPMD program for an nloc-samples-per-core shard."""
    if n_iters is None:
        n_iters = N_ITERS
    assert nloc % (P * JG) == 0
    nt = nloc // P  # sample tiles per core
    ng = nt // JG  # DMA groups
    ntot = nloc * ncores
    c1 = (1.0 - EPS) / (ntot - 1.0)

    nc = bacc.Bacc(
        "TRN2",
        target_bir_lowering=False,
        debug=False,
        enable_asserts=False,
        num_devices=ncores,
    )
    x_d = nc.dram_tensor("x", [nloc, C], F32, kind="ExternalInput").ap()
    # version tag: its LENGTH is bumped per kernel revision so the axon
    # terminal's HLO-keyed executable cache cannot return a stale NEFF.
    vt_d = nc.dram_tensor("vtag", [1, VTAG_LEN], F32, kind="ExternalInput").ap()
    out_d = nc.dram_tensor("out", [nloc, C], F32, kind="ExternalOutput").ap()
    np_consts = make_consts()
    cd = {}
    for name in CONST_NAMES:
        v = np_consts[name]
        dt = BF16 if v.dtype.name == "bfloat16" else F32
        cd[name] = nc.dram_tensor(name, list(v.shape), dt, kind="ExternalInput").ap()

    x_v = x_d.rearrange("(g p j) c -> g p j c", p=P, j=JG)
    out_v = out_d.rearrange("(g j p) c -> g p j c", p=P, j=JG)

    with tile.TileContext(nc) as tc:
        with (
            tc.tile_pool(name="const", bufs=1) as cpool,
            tc.tile_pool(name="xa", bufs=1) as xapool,
            tc.tile_pool(name="xt", bufs=1) as xtpool,
            tc.tile_pool(name="wk", bufs=2) as wpool,
            tc.tile_pool(name="wk1", bufs=1) as wpool1,
            tc.tile_pool(name="sm", bufs=1) as spool,
            tc.tile_pool(name="stg", bufs=STG_BUFS) as stpool,
            tc.tile_pool(name="psg", bufs=1, space="PSUM") as psg,
            tc.tile_pool(name="pst", bufs=PST_BUFS, space="PSUM") as pst,
            tc.tile_pool(name="ptr", bufs=PTR_BUFS, space="PSUM") as ptr,
            tc.tile_pool(name="dram", bufs=1, space="DRAM") as dpool,
        ):
            vt_sb = cpool.tile([1, VTAG_LEN], F32, tag="vtag", name="vtag")
            nc.sync.dma_start(out=vt_sb[:, :], in_=vt_d)
            # ---- constants to SBUF ----
            csb = {}
            for name in CONST_NAMES:
                shp = np_consts[name].shape
                dt = BF16 if np_consts[name].dtype.name == "bfloat16" else F32
                if shp == (C, C):
                    t = cpool.tile([P, 2, C], dt, tag=name, name=name)
                    nc.sync.dma_start(
                        out=t[:, :, :], in_=cd[name].rearrange("(r p) c -> p r c", p=P)
                    )
                else:
                    t = cpool.tile(list(shp), dt, tag=name, name=name)
                    nc.sync.dma_start(out=t[:, :], in_=cd[name])
                csb[name] = t

            # ---- phase 1: stream x; convert to bf16; Gram ASAP ----
            # xb tiles stay resident so the PE transposes have no deadline and
            # can fill the AllReduce / Newton-iteration windows.
            ng_def = min(NG_DEF, ng - 1)
            n_rot = ng - ng_def  # groups whose transposes are inline
            xabufs = [
                xapool.tile([P, JG, C], F32, tag=f"xa{j}", name=f"xa{j}")
                for j in range(NXA)
            ]
            nxb = min(NXB, n_rot) + ng_def
            xbbufs = [
                xapool.tile([P, JG, C + 1], BF16, tag=f"xb{j}", name=f"xb{j}")
                for j in range(nxb)
            ]
            for j in range(nxb):
                nc.vector.memset(xbbufs[j][:, :, C : C + 1], 1.0)

            def xb_of(g):
                if g >= n_rot:
                    return xbbufs[min(NXB, n_rot) + (g - n_rot)]
                return xbbufs[g % min(NXB, n_rot)]
            xt = [
                xtpool.tile([P, nloc], BF16, tag=f"xt{cb}", name=f"xt{cb}")
                for cb in range(2)
            ]
            pg = [
                psg.tile([P, C + 1], F32, tag=f"g{cb}", name=f"g{cb}")
                for cb in range(2)
            ]

            tp_inline = n_rot * JG
            tp_queue = list(range(tp_inline, nt))

            def emit_tp(ts):
                xsq = xb_of(ts // JG)[:, ts % JG, :]
                for cb in range(2):
                    pt = pst.tile([P, P], BF16, tag="tp", name="tp")
                    nc.tensor.transpose(
                        pt[:, :],
                        xsq[:, cb * P : (cb + 1) * P],
                        csb["eye128b"][:, :],
                    )
                    if cb == 0:
                        nc.vector.tensor_copy(
                            xt[cb][:, ts * P : (ts + 1) * P], pt[:, :]
                        )
                    else:
                        nc.scalar.copy(
                            out=xt[cb][:, ts * P : (ts + 1) * P], in_=pt[:, :]
                        )

            def emit_tp_some(k):
                for _ in range(min(k, len(tp_queue))):
                    emit_tp(tp_queue.pop(0))

            for g in range(ng):
                xa = xabufs[g % NXA]
                xb = xb_of(g)
                nc.sync.dma_start(out=xa[:, :, :], in_=x_v[g])
                for jj in range(JG):
                    ts = g * JG + jj
                    # fp32 -> bf16 conversion, split across DVE and GpSimd
                    eng = nc.vector if jj % 2 == 0 else nc.gpsimd
                    eng.tensor_copy(xb[:, jj, 0:C], xa[:, jj, :])
                    xs = xb[:, jj, :]
                    first = ts == 0
                    last = ts == nt - 1
                    for cb in range(2 if do_gram else 0):
                        nc.tensor.matmul(
                            pg[cb][:, :],
                            lhsT=xs[:, cb * P : (cb + 1) * P],
                            rhs=xs[:, :],
                            start=first,
                            stop=last,
                        )
                    if do_tp and ts < tp_inline:
                        emit_tp(ts)

            # ---- phase 2: AllReduce [G | s] ----
            gstage = spool.tile([P, 2, C + 1], F32, tag="gstage", name="gstage")
            for cb in range(2 if do_gram else 0):
                nc.vector.tensor_copy(gstage[:, cb, :], pg[cb][:, :])
            cc_in = dpool.tile([2 * P, C + 1], F32, tag="ccin", name="ccin")
            cc_out = dpool.tile([2 * P, C + 1], F32, tag="ccout", name="ccout")
            nc.sync.dma_start(
                out=cc_in[:, :].rearrange("(r p) f -> p r f", p=P), in_=gstage[:, :, :]
            )
            if ncores > 1:
                nc.gpsimd.collective_compute(
                    "AllReduce",
                    AL.add,
                    replica_groups=[list(range(ncores))],
                    ins=[cc_in[:, :].opt()],
                    outs=[cc_out[:, :].opt()],
                )
            else:
                # single-core build (cost-model timeline): AR degenerates to copy
                nc.sync.dma_start(out=cc_out[:, :], in_=cc_in[:, :])
            gsum = spool.tile([P, 2, C + 1], F32, tag="gsum", name="gsum")
            nc.sync.dma_start(
                out=gsum[:, :, :], in_=cc_out[:, :].rearrange("(r p) f -> p r f", p=P)
            )


            if do_tp:
                emit_tp_some(10)
            # ---- phase 3: A = c1*G - (c1/n) s s^T + eps I ; W0, V0 ----
            # s^T row via PE transpose of the two halves of s (through bf16 is
            # not acceptable here -> use fp32 transpose, it is tiny)
            st = spool.tile([1, C], F32, tag="st", name="st")
            eye128f = cpool.tile([P, P], F32, tag="eyef", name="eyef")
            # build fp32 identity once from the bf16 one (exact values)
            nc.vector.tensor_copy(eye128f[:, :], csb["eye128b"][:, :])
            for rb in range(2):
                pt = ptr.tile([1, P], F32, tag="tq", name="tq")
                nc.tensor.transpose(
                    pt[:, :], gsum[:, rb, C : C + 1], eye128f[:, :]
                )
                nc.vector.tensor_copy(st[0:1, rb * P : (rb + 1) * P], pt[:, :])

            A = spool.tile([P, 2, C], F32, tag="A", name="A")
            t1 = spool.tile([P, 2, C], F32, tag="t1", name="t1")
            t2 = spool.tile([P, 2, C], F32, tag="t2", name="t2")
            W = wpool.tile([P, 2, C], F32, tag="W", name="W")
            V = wpool.tile([P, 2, C], F32, tag="V", name="V")
            for rb in range(2):
                pss = ptr.tile([P, C], F32, tag="tq", name="tq")
                nc.tensor.matmul(
                    pss[:, :],
                    lhsT=st[0:1, rb * P : (rb + 1) * P],
                    rhs=st[0:1, :],
                    start=True,
                    stop=True,
                )
                nc.vector.scalar_tensor_tensor(
                    t1[:, rb, :], pss[:, :], c1 / ntot, csb["epsi"][:, rb, :],
                    AL.mult, AL.subtract,
                )
                nc.vector.scalar_tensor_tensor(
                    A[:, rb, :], gsum[:, rb, 0:C], c1, t1[:, rb, :],
                    AL.mult, AL.subtract,
                )
                nc.vector.scalar_tensor_tensor(
                    t2[:, rb, :], A[:, rb, :], 1.0, csb["ml"][:, rb, :],
                    AL.mult, AL.mult,
                )
                nc.vector.scalar_tensor_tensor(
                    W[:, rb, :], t2[:, rb, :], -1.0, csb["c15"][:, rb, :],
                    AL.mult, AL.add,
                )
                nc.vector.scalar_tensor_tensor(
                    t2[:, rb, 0:C], A[:, rb, :], 1.0, csb["mu"][:, rb, :],
                    AL.mult, AL.mult,
                )
                nc.vector.scalar_tensor_tensor(
                    V[:, rb, :], t2[:, rb, 0:C], -1.0, csb["c15"][:, rb, :],
                    AL.mult, AL.add,
                )

            # ---- phase 4: Newton iteration for the inverse Cholesky factor ----
            # early iterations run the matmuls in bf16 (quadratic convergence
            # washes out rounding); the last N_FP32 iterations polish in fp32.
            n_bf = max(0, n_iters - N_FP32)
            Ab = spool.tile([P, 2, C], BF16, tag="Ab", name="Ab")
            if n_bf > 0:
                for rb in range(2):
                    nc.vector.tensor_copy(Ab[:, rb, :], A[:, rb, :])
            for it in range(n_iters):
                bf = it < n_bf
                dt_it = BF16 if bf else F32
                A_it = Ab if bf else A
                if bf and it == 0:
                    # round the fp32 seeds once
                    Wb = wpool1.tile([P, 2, C], BF16, tag="Wb", name="Wb")
                    Vb0 = wpool1.tile([P, 2, C], BF16, tag="Vb0", name="Vb0")
                    for rb in range(2):
                        nc.vector.tensor_copy(Wb[:, rb, :], W[:, rb, :])
                        nc.vector.tensor_copy(Vb0[:, rb, :], V[:, rb, :])
                    W, V = Wb, Vb0
                if not bf and it == n_bf and n_bf > 0:
                    # promote back to fp32 for the polishing iterations
                    Wf = wpool.tile([P, 2, C], F32, tag="W", name="W")
                    Vf = wpool.tile([P, 2, C], F32, tag="V", name="V")
                    for rb in range(2):
                        nc.vector.tensor_copy(Wf[:, rb, :], W[:, rb, :])
                        nc.vector.tensor_copy(Vf[:, rb, :], V[:, rb, :])
                    W, V = Wf, Vf
                if do_tp:
                    emit_tp_some(4)
                Pm = wpool.tile([P, 2, C], dt_it, tag="Pm", name="Pm")
                for rb in range(2):
                    pp = ptr.tile([P, C], F32, tag="tq", name="tq")
                    for kk in range(2):
                        nc.tensor.matmul(
                            pp[:, :],
                            lhsT=A_it[:, kk, rb * P : (rb + 1) * P],
                            rhs=V[:, kk, :],
                            start=(kk == 0),
                            stop=(kk == 1),
                        )
                    nc.vector.tensor_copy(Pm[:, rb, :], pp[:, :])
                if do_tp:
                    emit_tp_some(4)
                tmpT = wpool.tile([P, 2, C], dt_it, tag="tT", name="tT")
                u = wpool1.tile([P, 2, C], F32, tag="u", name="u")
                for rb in range(2):
                    pr = ptr.tile([P, C], F32, tag="tq", name="tq")
                    for kk in range(2):
                        nc.tensor.matmul(
                            pr[:, :],
                            lhsT=Pm[:, kk, rb * P : (rb + 1) * P],
                            rhs=V[:, kk, :],
                            start=(kk == 0),
                            stop=(kk == 1),
                        )
                    nc.vector.scalar_tensor_tensor(
                        u[:, rb, :], pr[:, :], 1.0, csb["mu"][:, rb, :],
                        AL.mult, AL.mult,
                    )
                    nc.vector.scalar_tensor_tensor(
                        tmpT[:, rb, :], u[:, rb, :], 1.0, csb["ih"][:, rb, :],
                        AL.mult, AL.subtract,
                    )
                if do_tp:
                    emit_tp_some(4)
                Wn = wpool.tile([P, 2, C], dt_it, tag="W2" if bf else "W", name="Wn")
                Vn = wpool.tile([P, 2, C], dt_it, tag="V2" if bf else "V", name="Vn")
                for rb in range(2):
                    pv = ptr.tile([P, C], F32, tag="tq", name="tq")
                    for kk in range(2):
                        nc.tensor.matmul(
                            pv[:, :],
                            lhsT=W[:, kk, rb * P : (rb + 1) * P],
                            rhs=tmpT[:, kk, :],
                            start=(kk == 0),
                            stop=(kk == 1),
                        )
                    nc.vector.scalar_tensor_tensor(
                        Vn[:, rb, :], pv[:, :], -1.0, V[:, rb, :], AL.mult, AL.add
                    )
                    pw = ptr.tile([P, C], F32, tag="tq", name="tq")
                    for kk in range(2):
                        nc.tensor.matmul(
                            pw[:, :],
                            lhsT=tmpT[:, kk, rb * P : (rb + 1) * P],
                            rhs=W[:, kk, :],
                            start=(kk == 0),
                            stop=(kk == 1),
                        )
                    nc.vector.scalar_tensor_tensor(
                        Wn[:, rb, :], pw[:, :], -1.0, W[:, rb, :], AL.mult, AL.add
                    )
                W, V = Wn, Vn

            # ---- phase 5: mean-correction row + bf16 V ----
            pmv = ptr.tile([1, C], F32, tag="tq", name="tq")
            for kk in range(2):
                nc.tensor.matmul(
                    pmv[:, :],
                    lhsT=gsum[:, kk, C : C + 1],
                    rhs=V[:, kk, :],
                    start=(kk == 0),
                    stop=(kk == 1),
                )
            mv = spool.tile([1, C], F32, tag="mv", name="mv")
            nc.vector.tensor_scalar_mul(mv[:, :], pmv[:, :], 1.0 / ntot)
            pmvb = ptr.tile([P, C], F32, tag="tq", name="tq")
            nc.tensor.matmul(
                pmvb[:, :], lhsT=csb["onesrow"][0:1, :], rhs=mv[0:1, :],
                start=True, stop=True,
            )
            mvb = spool.tile([P, C], F32, tag="mvb", name="mvb")
            nc.vector.tensor_copy(mvb[:, :], pmvb[:, :])
            Vb = spool.tile([P, 2, C], BF16, tag="Vb", name="Vb")
            for kk in range(2):
                nc.vector.tensor_copy(Vb[:, kk, :], V[:, kk, :])
            onesb = spool.tile([1, P], BF16, tag="onesb", name="onesb")
            nc.vector.tensor_copy(onesb[:, :], csb["onesrow"][:, :])
            mvnb = spool.tile([1, C], BF16, tag="mvnb", name="mvnb")
            nc.vector.tensor_scalar_mul(mvnb[:, :], mv[:, :], -1.0)

            if do_tp:
                emit_tp_some(len(tp_queue))
            # ---- phase 6: whiten out = x V - mvb, from resident x^T ----
            # HGT sample tiles per output store (big DMAs are critical)
            assert nt % HGT == 0
            assert HGT == JG
            out_sv = out_d.rearrange("(s p j) c -> s p j c", p=P, j=HGT)
            for st in range(nt // HGT if do_whiten else 0):
                osb = stpool.tile([P, HGT, C], F32, tag="osb", name="osb")
                for jj in range(HGT):
                    ts = st * HGT + jj
                    po = pst.tile([P, C], F32, tag="tp", name="tp")
                    for kk in range(2):
                        nc.tensor.matmul(
                            po[:, :],
                            lhsT=xt[kk][:, ts * P : (ts + 1) * P],
                            rhs=Vb[:, kk, :],
                            start=(kk == 0),
                            stop=False,
                        )
                    nc.tensor.matmul(
                        po[:, :], lhsT=onesb[0:1, :], rhs=mvnb[0:1, :],
                        start=False, stop=True,
                    )
                    if jj % 2 == 0:
                        nc.vector.tensor_copy(osb[:, jj, :], po[:, :])
                    else:
                        nc.scalar.copy(out=osb[:, jj, :], in_=po[:, :])
                nc.sync.dma_start(out=out_sv[st], in_=osb[:, :, :])

    nc.compile()
    return nc


_CACHE = {}


def _get_nc(nloc: int):
    if nloc not in _CACHE:
        _CACHE[nloc] = build(nloc)
    return _CACHE[nloc]


def kernel(**inputs) -> np.ndarray:
    x = np.ascontiguousarray(np.asarray(inputs["x"], dtype=np.float32))
    b, w, h, c = x.shape
    assert c == C
    n = b * w * h
    nloc = n // NCORES
    xf = x.reshape(n, C)
    consts = make_consts()
    in_maps = []
    for i in range(NCORES):
        m = {"x": xf[i * nloc : (i + 1) * nloc],
             "vtag": np.zeros((1, VTAG_LEN), np.float32)}
        m.update(consts)
        in_maps.append(m)
    nc = _get_nc(nloc)
    res = run_bass_kernel_spmd(nc, in_maps, core_ids=list(range(NCORES)))
    out = np.concatenate([res.results[i]["out"] for i in range(NCORES)], axis=0)
    return out.reshape(b, w, h, c)

